# revision 14
# baseline (speedup 1.0000x reference)
"""Bass/Trainium2 kernel for nn_BoltzmannGibbsMask.

Reference computation:
    m = jax.random.normal(jax.random.key(42), (N, N), f32)   # CONSTANT noise
    M = softmax(m / 0.5, axis=1)                             # row-wise Gibbs weights
    out = M @ x                                              # [N, D]

Key facts exploited here:
  * The noise (and therefore the whole softmax weight matrix W) is a
    compile-time constant - it does not depend on the inputs.  We fold
    softmax(2*m) on the host once (cached) and ship W to the device in fp16.
  * The environment pins jax_default_prng_impl=rbg, whose bits are
    backend-dependent.  The reference runs on the (axon/neuron) default
    backend, so we generate the noise with the *identical* jax call on the
    default backend - this reproduces the reference bits exactly.
  * Row-parallel sharding: core c owns output rows [c*1024, (c+1)*1024).
    Each core computes O_c = W_c @ x with W_c shipped pre-transposed and
    pre-blocked so every DMA is wide and every matmul lhsT tile is a
    natural [K=128, M=128] SBUF slice.  No cross-core communication.
  * fp16 matmul runs at full PE rate (1 cycle/row) with a 10-bit mantissa;
    measured output error vs the fp32 reference is ~3e-4 relative.
"""

import os
import tempfile

import numpy as np

N = 8192          # rows/cols of the mask, rows of x
D = 1024          # columns of x
NCORES = 8
RPC = N // NCORES   # output rows per core: 1024
P = 128             # partition dim
RT = RPC // P       # 8 row-tiles per core
JT = N // P         # 64 contraction tiles
INV_ALPHA = 2.0     # 1/0.5

_WT_CACHE_PATH = os.path.join(
    tempfile.gettempdir(), "bgm41351945125987_wt16_v1.npy"
)

_state = {"wt16": None, "nc": None}


def _fold_constant_weights() -> np.ndarray:
    """softmax(2*noise) folded on host, fp16, blocked per core for the device.

    Returns wt16 with shape [NCORES, RT, P(=j), JT, P(=r)] where
        wt16[c, rt, j, jt, r] = W[c*RPC + rt*P + r, jt*P + j]
    i.e. for each 128-row output tile the weights are stored contraction-major
    so each SBUF lhsT tile [j, r] is a contiguous 4KB-per-partition DMA.
    """
    if _state["wt16"] is not None:
        return _state["wt16"]

    wt16 = None
    try:
        if os.path.exists(_WT_CACHE_PATH):
            cand = np.load(_WT_CACHE_PATH)
            if cand.shape == (NCORES, RT, P, JT, P) and cand.dtype == np.float16:
                wt16 = cand
    except Exception:
        wt16 = None

    if wt16 is None:
        import jax
        import jax.numpy as jnp

        # Identical call path to the reference => identical rbg bits on the
        # default (neuron/axon) backend.
        key = jax.random.key(42)
        m = np.array(jax.random.normal(key, (N, N), dtype=jnp.float32))

        # Stable row softmax of (2*m), in float32 like the reference.
        mx = m.max(axis=1, keepdims=True)
        np.subtract(m, mx, out=m)
        np.multiply(m, INV_ALPHA, out=m)
        e = np.exp(m)
        del m
        s = e.sum(axis=1, keepdims=True, dtype=np.float32)
        np.divide(e, s, out=e)
        w16 = e.astype(np.float16)
        del e

        # [c, rt, r, jt, j] -> [c, rt, j, jt, r]
        wt16 = np.ascontiguousarray(
            w16.reshape(NCORES, RT, P, JT, P).transpose(0, 1, 4, 3, 2)
        )
        del w16
        try:
            fd, tmp = tempfile.mkstemp(
                dir=os.path.dirname(_WT_CACHE_PATH), suffix=".npy.tmp"
            )
            os.close(fd)
            np.save(tmp, wt16)
            os.replace(tmp + ".npy" if os.path.exists(tmp + ".npy") else tmp,
                       _WT_CACHE_PATH)
        except Exception:
            pass

    _state["wt16"] = wt16
    return wt16


def _build_program():
    """One NeuronCore's program (SPMD: same NEFF on all 8 cores).

    Inputs : wt  [RT, P, JT, P] fp16   (this core's blocked weight slice)
             x16 [N, D]        fp16   (replicated activations)
    Output : out [RPC, D]      fp32   (this core's output rows)
    """
    if _state["nc"] is not None:
        return _state["nc"]

    import concourse.tile as tile
    from concourse import bacc, mybir

    # Bacc (not raw Bass): its compile pipeline legalizes multi-wait
    # instructions (generate_event_semaphores / split waits) down to the
    # <=1-embedded-wait-per-instruction hardware constraint that walrus
    # enforces. Raw Bass + Tile emits 2-wait DMAs on slot reuse and fails
    # codegen with "Too many sync wait commands".
    nc = bacc.Bacc(
        "TRN2",
        target_bir_lowering=False,
        debug=False,
        num_devices=NCORES,
    )
    wt = nc.dram_tensor("wt", [RT, P, JT, P], mybir.dt.float16,
                        kind="ExternalInput")
    x16 = nc.dram_tensor("x16", [N, D], mybir.dt.float16, kind="ExternalInput")
    out = nc.dram_tensor("out", [RPC, D], mybir.dt.float32,
                         kind="ExternalOutput")

    HEAD = 3              # row-tiles computed while the x stream loads
    JC = 8                # j-tiles per W DMA chunk (256KB per DMA)

    with tile.TileContext(nc) as tc:
        with (
            tc.tile_pool(name="xpool", bufs=1) as xpool,
            tc.tile_pool(name="wpool", bufs=HEAD + 1) as wpool,
            tc.tile_pool(name="opool", bufs=2) as opool,
            tc.tile_pool(name="psum", bufs=HEAD + 1, space="PSUM") as ppool,
        ):
            # x resident in SBUF: [j_local, jt, d], 128KB/partition.
            x_sb = xpool.tile([P, JT, D], mybir.dt.float16)

            def load_x(jt):
                nc.sync.dma_start(
                    out=x_sb[:, jt, :], in_=x16[jt * P:(jt + 1) * P, :]
                )

            def load_w_chunk(w_sb, rt, c):
                nc.sync.dma_start(
                    out=w_sb[:, c * JC:(c + 1) * JC, :],
                    in_=wt[rt, :, c * JC:(c + 1) * JC, :],
                )

            def evict(rt, psum):
                o_sb = opool.tile([P, D], mybir.dt.float32)
                nc.scalar.copy(o_sb[:], psum[:])
                nc.sync.dma_start(
                    out=out[rt * P:(rt + 1) * P, :], in_=o_sb[:]
                )

            def mm(psum, w_sb, jt, first, last):
                for dh in range(2):
                    nc.tensor.matmul(
                        psum[:, dh * 512:(dh + 1) * 512],
                        w_sb[:, jt, :],
                        x_sb[:, jt, dh * 512:(dh + 1) * 512],
                        start=first,
                        stop=last,
                    )

            # --- head: row-tiles 0..HEAD-1 share the incoming x stream.
            # DMAs are issued in first-use order (W chunk + x tiles of that
            # chunk, then matmuls on them) so the PE starts within a few us
            # and 3 row-tiles per x tile keep it PE-bound while x loads.
            w_head = [wpool.tile([P, JT, P], mybir.dt.float16, tag="w",
                                 name=f"w_head{r}")
                      for r in range(HEAD)]
            ps_head = [ppool.tile([P, D], mybir.dt.float32, tag="ps",
                                  name=f"ps_head{r}")
                       for r in range(HEAD)]
            for c in range(JT // JC):
                # DMA issue order = first-use order: the W chunk the first
                # LDWEIGHTS needs, then x for the first j-tile (first
                # matmul's rhs), then the rest of the chunk's tiles.
                load_w_chunk(w_head[0], 0, c)
                load_x(c * JC)
                for r in range(1, HEAD):
                    load_w_chunk(w_head[r], r, c)
                for jt in range(c * JC + 1, (c + 1) * JC):
                    load_x(jt)
                for jt in range(c * JC, (c + 1) * JC):
                    for r in range(HEAD):
                        mm(ps_head[r], w_head[r], jt, jt == 0, jt == JT - 1)
            for r in range(HEAD):
                evict(r, ps_head[r])

            # --- steady state: one row-tile at a time, W prefetched one
            # row-tile ahead.
            for rt in range(HEAD, RT):
                w_sb = wpool.tile([P, JT, P], mybir.dt.float16, tag="w")
                for c in range(JT // JC):
                    load_w_chunk(w_sb, rt, c)
                psum = ppool.tile([P, D], mybir.dt.float32, tag="ps")
                if rt < RT - 1:
                    for jt in range(JT):
                        mm(psum, w_sb, jt, jt == 0, jt == JT - 1)
                    evict(rt, psum)
                else:
                    # Last row-tile: run the d0 half over all j first so its
                    # PSUM bank finishes mid-row-tile and its eviction (copy +
                    # store DMA) overlaps the d1 matmuls; the d1 tail is
                    # evicted in 256-col slices so copy and store pipeline.
                    o_sb = opool.tile([P, D], mybir.dt.float32)
                    for dh in range(2):
                        for jt in range(JT):
                            nc.tensor.matmul(
                                psum[:, dh * 512:(dh + 1) * 512],
                                w_sb[:, jt, :],
                                x_sb[:, jt, dh * 512:(dh + 1) * 512],
                                start=jt == 0,
                                stop=jt == JT - 1,
                            )
                        n_sl = 1 if dh == 0 else 2
                        sl = 512 // n_sl
                        for s in range(n_sl):
                            lo = dh * 512 + s * sl
                            nc.scalar.copy(
                                o_sb[:, lo:lo + sl], psum[:, lo:lo + sl]
                            )
                            nc.sync.dma_start(
                                out=out[rt * P:(rt + 1) * P, lo:lo + sl],
                                in_=o_sb[:, lo:lo + sl],
                            )

    nc.compile()  # Bacc passes: reg alloc + wait legalization (event sems)

    _state["nc"] = nc
    return nc


def kernel(x: np.ndarray, edge_index: np.ndarray = None, **_unused) -> np.ndarray:
    """Full-input / full-output entry point. edge_index is unused (as in the
    reference module)."""
    from concourse.bass_utils import run_bass_kernel_spmd

    wt16 = _fold_constant_weights()
    x16 = np.ascontiguousarray(np.asarray(x, dtype=np.float32).astype(np.float16))
    assert x16.shape == (N, D), f"unexpected x shape {x16.shape}"

    nc = _build_program()

    in_maps = [{"wt": wt16[c], "x16": x16} for c in range(NCORES)]
    core_ids = list(range(NCORES))

    trace_env = os.environ.get("BGM_TRACE", "0")
    trace = trace_env != "0"
    trace_cores = core_ids if trace_env == "all" else ([0] if trace else None)
    res = run_bass_kernel_spmd(
        nc,
        in_maps,
        core_ids,
        trace=trace,
        trace_cores=trace_cores,
    )
    kernel._last_results = res  # for test harness introspection

    outp = np.empty((N, D), dtype=np.float32)
    for c in range(NCORES):
        outp[c * RPC:(c + 1) * RPC, :] = res.results[c]["out"]
    return outp
